# revision 5
# baseline (speedup 1.0000x reference)
"""Trainium2 Bass kernel for nn_GammaModel (3-block Mamba-style model).

Sharding: data-parallel over batch. 8 cores x 4 samples each; all weights
replicated. Feature-major ([feature, token]) layout on device throughout:
 - all dense layers run on PE with the weight stationary,
 - the causal depthwise conv runs on PE as 8 PSUM-accumulated matmuls with
   host-diagonalized tap weights,
 - dA = exp(A[:,s] * delta) runs on ACT with per-partition scale,
 - B/C row broadcasts run on GPSIMD (partition_broadcast),
 - the selective scan itself is DVE tensor_tensor_scan (h = dA*h + dBu)
   along the free/time axis, one instruction per (state, sample).
"""

import sys

sys.path.insert(0, "/opt/trn_rl_repo")

import numpy as np
import ml_dtypes

from concourse import bacc, bass, mybir, tile
from concourse.bass_utils import run_bass_kernel_spmd

F32 = mybir.dt.float32
BF16 = mybir.dt.bfloat16
AF = mybir.ActivationFunctionType
ALU = mybir.AluOpType

# Model dims (hardcoded per problem spec)
NB = 3          # mamba blocks
B_FULL = 32     # full batch
NCORES = 8
BB = B_FULL // NCORES   # samples per core
L = 4096
T = BB * L      # tokens per core
DM = 32
DI = 128
DS = 12
DC = 8
DR = 2
CH = 512        # psum column chunk


def _build_nc():
    nc = bacc.Bacc(None, target_bir_lowering=False, debug=False)

    # ---- dram I/O ----
    xT_d = nc.dram_tensor("xT", (4, T), F32, kind="ExternalInput")
    fc0_wT_d = nc.dram_tensor("fc0_wT", (4, DM), F32, kind="ExternalInput")
    fc1_wT_d = nc.dram_tensor("fc1_wT", (DM, 2), F32, kind="ExternalInput")
    fc1_b_d = nc.dram_tensor("fc1_b", (2, 1), F32, kind="ExternalInput")
    lin_wT_d, lin_b_d, in_wT_d = [], [], []
    convd_d, conv_b_d, xproj_wT_d = [], [], []
    dt_wT_d, dt_b_d, A_d, Dp_d, out_wT_d = [], [], [], [], []
    for i in range(NB):
        lin_wT_d.append(nc.dram_tensor(f"lin_wT{i}", (DM, DM), F32, kind="ExternalInput"))
        lin_b_d.append(nc.dram_tensor(f"lin_b{i}", (DM, 1), F32, kind="ExternalInput"))
        in_wT_d.append(nc.dram_tensor(f"in_wT{i}", (DM, 2 * DI), F32, kind="ExternalInput"))
        convd_d.append(nc.dram_tensor(f"convd{i}", (DI, DC, DI), BF16, kind="ExternalInput"))
        conv_b_d.append(nc.dram_tensor(f"conv_b{i}", (DI, 1), F32, kind="ExternalInput"))
        xproj_wT_d.append(nc.dram_tensor(f"xproj_wT{i}", (DI, DR + 2 * DS), BF16, kind="ExternalInput"))
        dt_wT_d.append(nc.dram_tensor(f"dt_wT{i}", (DR, DI), BF16, kind="ExternalInput"))
        dt_b_d.append(nc.dram_tensor(f"dt_b{i}", (DI, 1), F32, kind="ExternalInput"))
        A_d.append(nc.dram_tensor(f"A{i}", (DI, DS), F32, kind="ExternalInput"))
        Dp_d.append(nc.dram_tensor(f"Dp{i}", (DI, 1), F32, kind="ExternalInput"))
        out_wT_d.append(nc.dram_tensor(f"out_wT{i}", (DI, DM), BF16, kind="ExternalInput"))
    out_d = nc.dram_tensor("out2", (2, BB), F32, kind="ExternalOutput")

    with tile.TileContext(nc) as tc:
        with (
            tc.tile_pool(name="w", bufs=1) as wp,
            tc.tile_pool(name="u", bufs=1) as up,
            tc.tile_pool(name="work", bufs=1) as pp,
            tc.tile_pool(name="small", bufs=2) as sp,
            tc.tile_pool(name="psum", bufs=1, space=bass.MemorySpace.PSUM) as psp,
        ):
            # ---- load weights (once) ----
            def wload(dram, shape, dtype, tag):
                t = wp.tile(shape, dtype, tag=tag)
                nc.sync.dma_start(t[:], dram[:])
                return t

            fc0_wT = wload(fc0_wT_d, (4, DM), F32, "fc0")
            fc1_wT = wload(fc1_wT_d, (DM, 2), F32, "fc1")
            fc1_b = wload(fc1_b_d, (2, 1), F32, "fc1b")
            lin_wT = [wload(lin_wT_d[i], (DM, DM), F32, f"linw{i}") for i in range(NB)]
            lin_b = [wload(lin_b_d[i], (DM, 1), F32, f"linb{i}") for i in range(NB)]
            in_wT = [wload(in_wT_d[i], (DM, 2 * DI), F32, f"inw{i}") for i in range(NB)]
            convd = [wload(convd_d[i], (DI, DC, DI), BF16, f"convd{i}") for i in range(NB)]
            conv_b = [wload(conv_b_d[i], (DI, 1), F32, f"convb{i}") for i in range(NB)]
            xproj_wT = [wload(xproj_wT_d[i], (DI, DR + 2 * DS), BF16, f"xpw{i}") for i in range(NB)]
            dt_wT = [wload(dt_wT_d[i], (DR, DI), BF16, f"dtw{i}") for i in range(NB)]
            dt_b = [wload(dt_b_d[i], (DI, 1), F32, f"dtb{i}") for i in range(NB)]
            A_t = [wload(A_d[i], (DI, DS), F32, f"A{i}") for i in range(NB)]
            Dp_t = [wload(Dp_d[i], (DI, 1), F32, f"Dp{i}") for i in range(NB)]
            out_wT = [wload(out_wT_d[i], (DI, DM), BF16, f"outw{i}") for i in range(NB)]

            u_a = nc.dram_tensor("u_dram_a", (DM, T), F32)
            u_b = nc.dram_tensor("u_dram_b", (DM, T), F32)
            ubufs = [u_a, u_b]

            # ---- embed: u0 = fc0_w_scaled @ xT  (+fc0_b folded == 0) ----
            for j in range(T // CH):
                xchunk = sp.tile((4, CH), F32, tag="xchunk")
                nc.sync.dma_start(xchunk[:], xT_d[:, j * CH:(j + 1) * CH])
                ps = psp.tile((DM, CH), F32, tag="pA")
                nc.tensor.matmul(ps[:], fc0_wT[:], xchunk[:])
                ustage = sp.tile((DM, CH), F32, tag="ustage")
                nc.scalar.copy(ustage[:], ps[:])
                nc.sync.dma_start(u_a[:, j * CH:(j + 1) * CH], ustage[:])

            # ---- blocks ----
            for i in range(NB):
                uin = ubufs[i % 2]
                uout = ubufs[(i + 1) % 2]
                for n in range(BB):
                    base = n * L
                    xhat = pp.tile((DI, L), BF16, tag="xhat")
                    zbuf = pp.tile((DI, L), BF16, tag="zbuf")
                    # lin -> tanh -> in_proj (x, z)
                    for j in range(L // CH):
                        gc = base + j * CH
                        lc = j * CH
                        uc = sp.tile((DM, CH), F32, tag="uc")
                        nc.sync.dma_start(uc[:], uin[:, gc:gc + CH])
                        lps = psp.tile((DM, CH), F32, tag="pA")
                        nc.tensor.matmul(lps[:], lin_wT[i][:], uc[:])
                        linc = sp.tile((DM, CH), F32, tag="linc")
                        nc.scalar.activation(linc[:], lps[:], AF.Tanh, bias=lin_b[i][:, 0:1])
                        xps = psp.tile((DI, CH), F32, tag="pX")
                        nc.tensor.matmul(xps[:], in_wT[i][:, 0:DI], linc[:])
                        zps = psp.tile((DI, CH), F32, tag="pZ")
                        nc.tensor.matmul(zps[:], in_wT[i][:, DI:2 * DI], linc[:])
                        nc.scalar.copy(xhat[:, lc:lc + CH], xps[:])
                        nc.vector.tensor_copy(zbuf[:, lc:lc + CH], zps[:])
                    # causal depthwise conv (PE, accumulated diag matmuls) + silu
                    xc = pp.tile((DI, L), BF16, tag="xc")
                    for j in range(L // CH):
                        lc = j * CH
                        cps = psp.tile((DI, CH), F32, tag="pC")
                        nc.tensor.matmul(cps[:], convd[i][:, DC - 1, :], xhat[:, lc:lc + CH],
                                         start=True, stop=False)
                        for k in range(DC - 1):
                            s = DC - 1 - k
                            last = (k == DC - 2)
                            if j == 0:
                                nc.tensor.matmul(cps[:, s:CH], convd[i][:, k, :],
                                                 xhat[:, 0:CH - s],
                                                 start=False, stop=last)
                            else:
                                nc.tensor.matmul(cps[:], convd[i][:, k, :],
                                                 xhat[:, lc - s:lc - s + CH],
                                                 start=False, stop=last)
                        nc.scalar.activation(xc[:, lc:lc + CH], cps[:], AF.Silu,
                                             bias=conv_b[i][:, 0:1])
                    # xproj -> dt/B/C rows
                    dtBC = pp.tile((DR + 2 * DS, L), BF16, tag="dtBC")
                    for j in range(L // CH):
                        lc = j * CH
                        pps = psp.tile((DR + 2 * DS, CH), F32, tag="pP")
                        nc.tensor.matmul(pps[:], xproj_wT[i][:], xc[:, lc:lc + CH])
                        nc.vector.tensor_copy(dtBC[:, lc:lc + CH], pps[:])
                    # delta = softplus(dt @ dt_w.T + dt_b) = ln(1 + exp(.))
                    # (no softplus table in this compiler build; args are tiny
                    #  so exp cannot overflow)
                    deltaT = pp.tile((DI, L), F32, tag="deltaT")
                    for j in range(L // CH):
                        lc = j * CH
                        dps = psp.tile((DI, CH), F32, tag="pD")
                        nc.tensor.matmul(dps[:], dt_wT[i][:], dtBC[0:DR, lc:lc + CH])
                        spe = sp.tile((DI, CH), F32, tag="spe")
                        nc.scalar.activation(spe[:], dps[:], AF.Exp,
                                             bias=dt_b[i][:, 0:1])
                        nc.vector.tensor_scalar_add(spe[:], spe[:], 1.0)
                        nc.scalar.activation(deltaT[:, lc:lc + CH], spe[:], AF.Ln)
                    # du = delta * x
                    du = pp.tile((DI, L), BF16, tag="du")
                    nc.vector.tensor_mul(du[:], deltaT[:], xc[:])
                    # selective scan over states
                    ybf = pp.tile((DI, L), BF16, tag="ybf")
                    for s in range(DS):
                        dA = pp.tile((DI, L), BF16, tag="dA")
                        nc.scalar.activation(dA[:], deltaT[:], AF.Exp,
                                             scale=A_t[i][:, s:s + 1])
                        browB = sp.tile((1, L), BF16, tag="browB")
                        nc.sync.dma_start(browB[:], dtBC[DR + s:DR + s + 1, :])
                        bcB = pp.tile((DI, L), BF16, tag="bcB")
                        nc.gpsimd.partition_broadcast(bcB[:], browB[0:1, :])
                        dBu = pp.tile((DI, L), BF16, tag="dBu")
                        nc.vector.tensor_mul(dBu[:], du[:], bcB[:])
                        h = pp.tile((DI, L), BF16, tag="h")
                        nc.vector.tensor_tensor_scan(h[:], dA[:], dBu[:], 0.0,
                                                     ALU.mult, ALU.add)
                        browC = sp.tile((1, L), BF16, tag="browC")
                        nc.sync.dma_start(browC[:], dtBC[DR + DS + s:DR + DS + s + 1, :])
                        bcC = pp.tile((DI, L), BF16, tag="bcC")
                        nc.gpsimd.partition_broadcast(bcC[:], browC[0:1, :])
                        if s == 0:
                            nc.vector.tensor_mul(ybf[:], h[:], bcC[:])
                        else:
                            hC = pp.tile((DI, L), BF16, tag="hC")
                            nc.vector.tensor_mul(hC[:], h[:], bcC[:])
                            nc.vector.tensor_add(ybf[:], ybf[:], hC[:])
                    # y = (x*Dp + y) * silu(z);  out = relu(out_w @ y)
                    y1 = pp.tile((DI, L), BF16, tag="y1")
                    nc.vector.scalar_tensor_tensor(y1[:], xc[:], Dp_t[i][:, 0:1], ybf[:],
                                                   ALU.mult, ALU.add)
                    sz = pp.tile((DI, L), BF16, tag="sz")
                    nc.scalar.activation(sz[:], zbuf[:], AF.Silu)
                    y2 = pp.tile((DI, L), BF16, tag="y2")
                    nc.vector.tensor_mul(y2[:], y1[:], sz[:])
                    for j in range(L // CH):
                        lc = j * CH
                        ops = psp.tile((DM, CH), F32, tag="pA")
                        nc.tensor.matmul(ops[:], out_wT[i][:], y2[:, lc:lc + CH])
                        ustage = sp.tile((DM, CH), F32, tag="ustage")
                        nc.scalar.activation(ustage[:], ops[:], AF.Relu)
                        nc.sync.dma_start(uout[:, base + lc:base + lc + CH], ustage[:])
            # ---- head: fc1 on last token of each sample ----
            ufin = ubufs[NB % 2]
            lastc = sp.tile((DM, BB), F32, tag="lastc")
            for n in range(BB):
                nc.sync.dma_start(lastc[:, n:n + 1], ufin[:, (n + 1) * L - 1:(n + 1) * L])
            fps = psp.tile((2, BB), F32, tag="pP")
            nc.tensor.matmul(fps[:], fc1_wT[:], lastc[:])
            outsb = sp.tile((2, BB), F32, tag="outsb")
            nc.scalar.activation(outsb[:], fps[:], AF.Relu, bias=fc1_b[:, 0:1])
            nc.sync.dma_start(out_d[:], outsb[:])

    nc.compile()
    return nc


_NC_CACHE = None


def _get_nc():
    global _NC_CACHE
    if _NC_CACHE is None:
        _NC_CACHE = _build_nc()
    return _NC_CACHE


def _prep_maps(x, fc0_w, fc0_b, lin_w, lin_b, in_w, conv_w, conv_b, xproj_w,
               dt_w, dt_b, A_log, D, out_w, fc1_w, fc1_b):
    f32 = np.float32
    bf16 = ml_dtypes.bfloat16
    start_max = np.max(np.asarray(x[:, :, 2], f32))
    scale = np.array([1.0 / 255.0, 1.0 / 255.0, 1.0 / start_max, 1.0], f32)
    fc0_wT = (np.asarray(fc0_w, f32) * scale[None, :]).T.copy()  # [4, 32]

    common = {
        "fc0_wT": fc0_wT.astype(f32),
        "fc1_wT": np.asarray(fc1_w, f32).T.copy(),
        "fc1_b": np.asarray(fc1_b, f32).reshape(2, 1),
    }
    for i in range(NB):
        common[f"lin_wT{i}"] = np.asarray(lin_w[i], f32).T.copy()
        common[f"lin_b{i}"] = np.asarray(lin_b[i], f32).reshape(DM, 1)
        common[f"in_wT{i}"] = np.asarray(in_w[i], f32).T.copy()
        cd = np.zeros((DI, DC, DI), f32)
        cw = np.asarray(conv_w[i], f32)
        for k in range(DC):
            cd[np.arange(DI), k, np.arange(DI)] = cw[:, k]
        common[f"convd{i}"] = cd.astype(bf16)
        common[f"conv_b{i}"] = np.asarray(conv_b[i], f32).reshape(DI, 1)
        common[f"xproj_wT{i}"] = np.asarray(xproj_w[i], f32).T.copy().astype(bf16)
        common[f"dt_wT{i}"] = np.asarray(dt_w[i], f32).T.copy().astype(bf16)
        common[f"dt_b{i}"] = np.asarray(dt_b[i], f32).reshape(DI, 1)
        common[f"A{i}"] = (-np.exp(np.asarray(A_log[i], f32))).astype(f32)
        common[f"Dp{i}"] = np.asarray(D[i], f32).reshape(DI, 1)
        common[f"out_wT{i}"] = np.asarray(out_w[i], f32).T.copy().astype(bf16)

    xf = np.asarray(x, f32)
    in_maps = []
    for c in range(NCORES):
        xc = xf[c * BB:(c + 1) * BB]          # [BB, L, 4]
        xTc = xc.reshape(BB * L, 4).T.copy()  # [4, T]
        m = dict(common)
        m["xT"] = np.ascontiguousarray(xTc)
        in_maps.append(m)
    return in_maps


def kernel(**inputs) -> np.ndarray:
    nc = _get_nc()
    in_maps = _prep_maps(**inputs)
    res = run_bass_kernel_spmd(nc, in_maps, list(range(NCORES)))
    out = np.zeros((B_FULL, 2), np.float32)
    for c in range(NCORES):
        out[c * BB:(c + 1) * BB] = res.results[c]["out2"].T
    return out


# revision 6
# speedup vs baseline: 3.3741x; 3.3741x over previous
"""Trainium2 Bass kernel for nn_GammaModel (3-block Mamba-style model).

Sharding: data-parallel over batch. 8 cores x 4 samples each; all weights
replicated. Feature-major ([feature, token]) layout on device throughout:
 - all dense layers run on PE with the weight stationary,
 - the causal depthwise conv runs on PE as 8 PSUM-accumulated matmuls with
   host-diagonalized tap weights,
 - dA = exp(A[:,s] * delta) runs on ACT with per-partition scale,
 - B/C row broadcasts run on GPSIMD (partition_broadcast),
 - the selective scan itself is DVE tensor_tensor_scan (h = dA*h + dBu)
   along the free/time axis, one instruction per (state, sample).
"""

import sys

sys.path.insert(0, "/opt/trn_rl_repo")

import numpy as np
import ml_dtypes

from concourse import bacc, bass, mybir, tile
from concourse.bass_utils import run_bass_kernel_spmd

F32 = mybir.dt.float32
BF16 = mybir.dt.bfloat16
AF = mybir.ActivationFunctionType
ALU = mybir.AluOpType

# Model dims (hardcoded per problem spec)
NB = 3          # mamba blocks
B_FULL = 32     # full batch
NCORES = 8
BB = B_FULL // NCORES   # samples per core
L = 4096
T = BB * L      # tokens per core
DM = 32
DI = 128
DS = 12
DC = 8
DR = 2
CH = 512        # psum column chunk


def _build_nc():
    nc = bacc.Bacc(None, target_bir_lowering=False, debug=False)

    # ---- dram I/O ----
    xT_d = nc.dram_tensor("xT", (4, T), F32, kind="ExternalInput")
    fc0_wT_d = nc.dram_tensor("fc0_wT", (4, DM), F32, kind="ExternalInput")
    fc1_wT_d = nc.dram_tensor("fc1_wT", (DM, 2), F32, kind="ExternalInput")
    fc1_b_d = nc.dram_tensor("fc1_b", (2, 1), F32, kind="ExternalInput")
    lin_wT_d, lin_b_d, in_wT_d = [], [], []
    convd_d, conv_b_d, xproj_wT_d = [], [], []
    dt_wT_d, dt_b_d, A_d, Dp_d, out_wT_d = [], [], [], [], []
    for i in range(NB):
        lin_wT_d.append(nc.dram_tensor(f"lin_wT{i}", (DM, DM), F32, kind="ExternalInput"))
        lin_b_d.append(nc.dram_tensor(f"lin_b{i}", (DM, 1), F32, kind="ExternalInput"))
        in_wT_d.append(nc.dram_tensor(f"in_wT{i}", (DM, 2 * DI), F32, kind="ExternalInput"))
        convd_d.append(nc.dram_tensor(f"convd{i}", (DI, DC, DI), BF16, kind="ExternalInput"))
        conv_b_d.append(nc.dram_tensor(f"conv_b{i}", (DI, 1), F32, kind="ExternalInput"))
        xproj_wT_d.append(nc.dram_tensor(f"xproj_wT{i}", (DI, DR + 2 * DS), BF16, kind="ExternalInput"))
        dt_wT_d.append(nc.dram_tensor(f"dt_wT{i}", (DR, DI), BF16, kind="ExternalInput"))
        dt_b_d.append(nc.dram_tensor(f"dt_b{i}", (DI, 1), F32, kind="ExternalInput"))
        A_d.append(nc.dram_tensor(f"A{i}", (DI, DS), F32, kind="ExternalInput"))
        Dp_d.append(nc.dram_tensor(f"Dp{i}", (DI, 1), F32, kind="ExternalInput"))
        out_wT_d.append(nc.dram_tensor(f"out_wT{i}", (DI, DM), BF16, kind="ExternalInput"))
    out_d = nc.dram_tensor("out2", (2, BB), F32, kind="ExternalOutput")

    with tile.TileContext(nc) as tc:
        with (
            tc.tile_pool(name="w", bufs=1) as wp,
            tc.tile_pool(name="u", bufs=1) as up,
            tc.tile_pool(name="work", bufs=1) as pp,
            tc.tile_pool(name="small", bufs=2) as sp,
            tc.tile_pool(name="psum", bufs=1, space=bass.MemorySpace.PSUM) as psp,
        ):
            # ---- load weights (once) ----
            def wload(dram, shape, dtype, tag):
                t = wp.tile(shape, dtype, tag=tag)
                nc.sync.dma_start(t[:], dram[:])
                return t

            fc0_wT = wload(fc0_wT_d, (4, DM), F32, "fc0")
            fc1_wT = wload(fc1_wT_d, (DM, 2), F32, "fc1")
            fc1_b = wload(fc1_b_d, (2, 1), F32, "fc1b")
            lin_wT = [wload(lin_wT_d[i], (DM, DM), F32, f"linw{i}") for i in range(NB)]
            lin_b = [wload(lin_b_d[i], (DM, 1), F32, f"linb{i}") for i in range(NB)]
            in_wT = [wload(in_wT_d[i], (DM, 2 * DI), F32, f"inw{i}") for i in range(NB)]
            convd = [wload(convd_d[i], (DI, DC, DI), BF16, f"convd{i}") for i in range(NB)]
            conv_b = [wload(conv_b_d[i], (DI, 1), F32, f"convb{i}") for i in range(NB)]
            xproj_wT = [wload(xproj_wT_d[i], (DI, DR + 2 * DS), BF16, f"xpw{i}") for i in range(NB)]
            dt_wT = [wload(dt_wT_d[i], (DR, DI), BF16, f"dtw{i}") for i in range(NB)]
            dt_b = [wload(dt_b_d[i], (DI, 1), F32, f"dtb{i}") for i in range(NB)]
            A_t = [wload(A_d[i], (DI, DS), F32, f"A{i}") for i in range(NB)]
            Dp_t = [wload(Dp_d[i], (DI, 1), F32, f"Dp{i}") for i in range(NB)]
            out_wT = [wload(out_wT_d[i], (DI, DM), BF16, f"outw{i}") for i in range(NB)]

            u_a = nc.dram_tensor("u_dram_a", (DM, T), F32)
            u_b = nc.dram_tensor("u_dram_b", (DM, T), F32)
            ubufs = [u_a, u_b]

            # ---- embed: u0 = fc0_w_scaled @ xT  (+fc0_b folded == 0) ----
            for j in range(T // CH):
                xchunk = sp.tile((4, CH), F32, tag="xchunk")
                nc.sync.dma_start(xchunk[:], xT_d[:, j * CH:(j + 1) * CH])
                ps = psp.tile((DM, CH), F32, tag="pA")
                nc.tensor.matmul(ps[:], fc0_wT[:], xchunk[:])
                ustage = sp.tile((DM, CH), F32, tag="ustage")
                nc.scalar.copy(ustage[:], ps[:])
                nc.sync.dma_start(u_a[:, j * CH:(j + 1) * CH], ustage[:])

            # ---- blocks ----
            for i in range(NB):
                uin = ubufs[i % 2]
                uout = ubufs[(i + 1) % 2]
                for n in range(BB):
                    base = n * L
                    xhat = pp.tile((DI, L), BF16, tag="xhat")
                    zbuf = pp.tile((DI, L), BF16, tag="zbuf")
                    # lin -> tanh -> in_proj (x, z)
                    for j in range(L // CH):
                        gc = base + j * CH
                        lc = j * CH
                        uc = sp.tile((DM, CH), F32, tag="uc")
                        nc.sync.dma_start(uc[:], uin[:, gc:gc + CH])
                        lps = psp.tile((DM, CH), F32, tag="pA")
                        nc.tensor.matmul(lps[:], lin_wT[i][:], uc[:])
                        linc = sp.tile((DM, CH), F32, tag="linc")
                        nc.scalar.activation(linc[:], lps[:], AF.Tanh, bias=lin_b[i][:, 0:1])
                        xps = psp.tile((DI, CH), F32, tag="pX")
                        nc.tensor.matmul(xps[:], in_wT[i][:, 0:DI], linc[:])
                        zps = psp.tile((DI, CH), F32, tag="pZ")
                        nc.tensor.matmul(zps[:], in_wT[i][:, DI:2 * DI], linc[:])
                        nc.scalar.copy(xhat[:, lc:lc + CH], xps[:])
                        nc.vector.tensor_copy(zbuf[:, lc:lc + CH], zps[:])
                    # causal depthwise conv (PE, accumulated diag matmuls) + silu
                    xc = pp.tile((DI, L), BF16, tag="xc")
                    for j in range(L // CH):
                        lc = j * CH
                        cps = psp.tile((DI, CH), F32, tag="pC")
                        nc.tensor.matmul(cps[:], convd[i][:, DC - 1, :], xhat[:, lc:lc + CH],
                                         start=True, stop=False)
                        for k in range(DC - 1):
                            s = DC - 1 - k
                            last = (k == DC - 2)
                            if j == 0:
                                nc.tensor.matmul(cps[:, s:CH], convd[i][:, k, :],
                                                 xhat[:, 0:CH - s],
                                                 start=False, stop=last)
                            else:
                                nc.tensor.matmul(cps[:], convd[i][:, k, :],
                                                 xhat[:, lc - s:lc - s + CH],
                                                 start=False, stop=last)
                        nc.scalar.activation(xc[:, lc:lc + CH], cps[:], AF.Silu,
                                             bias=conv_b[i][:, 0:1])
                    # xproj -> dt/B/C rows
                    dtBC = pp.tile((DR + 2 * DS, L), BF16, tag="dtBC")
                    for j in range(L // CH):
                        lc = j * CH
                        pps = psp.tile((DR + 2 * DS, CH), F32, tag="pP")
                        nc.tensor.matmul(pps[:], xproj_wT[i][:], xc[:, lc:lc + CH])
                        nc.vector.tensor_copy(dtBC[:, lc:lc + CH], pps[:])
                    # delta = softplus(dt @ dt_w.T + dt_b) = ln(1 + exp(.))
                    # (no softplus table in this compiler build; args are tiny
                    #  so exp cannot overflow)
                    deltaT = pp.tile((DI, L), F32, tag="deltaT")
                    for j in range(L // CH):
                        lc = j * CH
                        dps = psp.tile((DI, CH), F32, tag="pD")
                        nc.tensor.matmul(dps[:], dt_wT[i][:], dtBC[0:DR, lc:lc + CH])
                        spe = sp.tile((DI, CH), F32, tag="spe")
                        nc.scalar.activation(spe[:], dps[:], AF.Exp,
                                             bias=dt_b[i][:, 0:1])
                        nc.vector.tensor_scalar_add(spe[:], spe[:], 1.0)
                        nc.scalar.activation(deltaT[:, lc:lc + CH], spe[:], AF.Ln)
                    # du = delta * x
                    du = pp.tile((DI, L), BF16, tag="du")
                    nc.vector.tensor_mul(du[:], deltaT[:], xc[:])
                    # selective scan over states
                    ybf = pp.tile((DI, L), BF16, tag="ybf")
                    for s in range(DS):
                        dA = pp.tile((DI, L), BF16, tag="dA")
                        nc.scalar.activation(dA[:], deltaT[:], AF.Exp,
                                             scale=A_t[i][:, s:s + 1])
                        browB = sp.tile((1, L), BF16, tag="browB")
                        nc.sync.dma_start(browB[:], dtBC[DR + s:DR + s + 1, :])
                        bcB = pp.tile((DI, L), BF16, tag="bcB")
                        nc.gpsimd.partition_broadcast(bcB[:], browB[0:1, :])
                        dBu = pp.tile((DI, L), BF16, tag="dBu")
                        nc.vector.tensor_mul(dBu[:], du[:], bcB[:])
                        h = pp.tile((DI, L), BF16, tag="h")
                        nc.vector.tensor_tensor_scan(h[:], dA[:], dBu[:], 0.0,
                                                     ALU.mult, ALU.add)
                        browC = sp.tile((1, L), BF16, tag="browC")
                        nc.sync.dma_start(browC[:], dtBC[DR + DS + s:DR + DS + s + 1, :])
                        bcC = pp.tile((DI, L), BF16, tag="bcC")
                        nc.gpsimd.partition_broadcast(bcC[:], browC[0:1, :])
                        if s == 0:
                            nc.vector.tensor_mul(ybf[:], h[:], bcC[:])
                        else:
                            hC = pp.tile((DI, L), BF16, tag="hC")
                            nc.vector.tensor_mul(hC[:], h[:], bcC[:])
                            nc.vector.tensor_add(ybf[:], ybf[:], hC[:])
                    # y = (x*Dp + y) * silu(z);  out = relu(out_w @ y)
                    y1 = pp.tile((DI, L), BF16, tag="y1")
                    nc.vector.scalar_tensor_tensor(y1[:], xc[:], Dp_t[i][:, 0:1], ybf[:],
                                                   ALU.mult, ALU.add)
                    sz = pp.tile((DI, L), BF16, tag="sz")
                    nc.scalar.activation(sz[:], zbuf[:], AF.Silu)
                    y2 = pp.tile((DI, L), BF16, tag="y2")
                    nc.vector.tensor_mul(y2[:], y1[:], sz[:])
                    for j in range(L // CH):
                        lc = j * CH
                        ops = psp.tile((DM, CH), F32, tag="pA")
                        nc.tensor.matmul(ops[:], out_wT[i][:], y2[:, lc:lc + CH])
                        ustage = sp.tile((DM, CH), F32, tag="ustage")
                        nc.scalar.activation(ustage[:], ops[:], AF.Relu)
                        nc.sync.dma_start(uout[:, base + lc:base + lc + CH], ustage[:])
            # ---- head: fc1 on last token of each sample ----
            ufin = ubufs[NB % 2]
            lastc = sp.tile((DM, BB), F32, tag="lastc")
            for n in range(BB):
                nc.sync.dma_start(lastc[:, n:n + 1], ufin[:, (n + 1) * L - 1:(n + 1) * L])
            fps = psp.tile((2, BB), F32, tag="pP")
            nc.tensor.matmul(fps[:], fc1_wT[:], lastc[:])
            outsb = sp.tile((2, BB), F32, tag="outsb")
            nc.scalar.activation(outsb[:], fps[:], AF.Relu, bias=fc1_b[:, 0:1])
            nc.sync.dma_start(out_d[:], outsb[:])

    nc.compile()
    return nc


_NC_CACHE = None


def _get_nc():
    global _NC_CACHE
    if _NC_CACHE is None:
        _NC_CACHE = _build_nc()
    return _NC_CACHE


def _prep_maps(x, fc0_w, fc0_b, lin_w, lin_b, in_w, conv_w, conv_b, xproj_w,
               dt_w, dt_b, A_log, D, out_w, fc1_w, fc1_b):
    f32 = np.float32
    bf16 = ml_dtypes.bfloat16
    start_max = np.max(np.asarray(x[:, :, 2], f32))
    scale = np.array([1.0 / 255.0, 1.0 / 255.0, 1.0 / start_max, 1.0], f32)
    fc0_wT = (np.asarray(fc0_w, f32) * scale[None, :]).T.copy()  # [4, 32]

    common = {
        "fc0_wT": fc0_wT.astype(f32),
        "fc1_wT": np.asarray(fc1_w, f32).T.copy(),
        "fc1_b": np.asarray(fc1_b, f32).reshape(2, 1),
    }
    for i in range(NB):
        common[f"lin_wT{i}"] = np.asarray(lin_w[i], f32).T.copy()
        common[f"lin_b{i}"] = np.asarray(lin_b[i], f32).reshape(DM, 1)
        common[f"in_wT{i}"] = np.asarray(in_w[i], f32).T.copy()
        cd = np.zeros((DI, DC, DI), f32)
        cw = np.asarray(conv_w[i], f32)
        for k in range(DC):
            cd[np.arange(DI), k, np.arange(DI)] = cw[:, k]
        common[f"convd{i}"] = cd.astype(bf16)
        common[f"conv_b{i}"] = np.asarray(conv_b[i], f32).reshape(DI, 1)
        common[f"xproj_wT{i}"] = np.asarray(xproj_w[i], f32).T.copy().astype(bf16)
        common[f"dt_wT{i}"] = np.asarray(dt_w[i], f32).T.copy().astype(bf16)
        common[f"dt_b{i}"] = np.asarray(dt_b[i], f32).reshape(DI, 1)
        common[f"A{i}"] = (-np.exp(np.asarray(A_log[i], f32))).astype(f32)
        common[f"Dp{i}"] = np.asarray(D[i], f32).reshape(DI, 1)
        common[f"out_wT{i}"] = np.asarray(out_w[i], f32).T.copy().astype(bf16)

    xf = np.asarray(x, f32)
    in_maps = []
    for c in range(NCORES):
        xc = xf[c * BB:(c + 1) * BB]          # [BB, L, 4]
        xTc = xc.reshape(BB * L, 4).T.copy()  # [4, T]
        m = dict(common)
        m["xT"] = np.ascontiguousarray(xTc)
        in_maps.append(m)
    return in_maps


_RUNNER_CACHE = None


def _get_runner():
    """Build (once) a cached jitted SPMD runner equivalent to
    bass2jax.run_bass_via_pjrt, so repeat kernel() calls skip retracing."""
    global _RUNNER_CACHE
    if _RUNNER_CACHE is not None:
        return _RUNNER_CACHE
    import jax
    from jax.sharding import Mesh, PartitionSpec
    from jax.experimental.shard_map import shard_map
    from concourse import bass2jax, mybir as _mybir

    nc = _get_nc()
    bass2jax.install_neuronx_cc_hook()
    partition_name = nc.partition_id_tensor.name if nc.partition_id_tensor else None
    in_names, out_names, out_avals, zero_outs = [], [], [], []
    for alloc in nc.m.functions[0].allocations:
        if not isinstance(alloc, _mybir.MemoryLocationSet):
            continue
        name = alloc.memorylocations[0].name
        if alloc.kind == "ExternalInput":
            if name != partition_name:
                in_names.append(name)
        elif alloc.kind == "ExternalOutput":
            shape = tuple(alloc.tensor_shape)
            dtype = _mybir.dt.np(alloc.dtype)
            out_avals.append(jax.core.ShapedArray(shape, dtype))
            out_names.append(name)
            zero_outs.append(np.zeros(shape, dtype))
    n_params = len(in_names)
    n_outs = len(out_avals)
    all_in = list(in_names) + list(out_names)
    if partition_name is not None:
        all_in.append(partition_name)

    def _body(*args):
        operands = list(args)
        if partition_name is not None:
            operands.append(bass2jax.partition_id_tensor())
        outs = bass2jax._bass_exec_p.bind(
            *operands,
            out_avals=tuple(out_avals),
            in_names=tuple(all_in),
            out_names=tuple(out_names),
            lowering_input_output_aliases=(),
            sim_require_finite=True,
            sim_require_nnan=True,
            nc=nc,
        )
        return tuple(outs)

    devices = jax.devices()[:NCORES]
    mesh = Mesh(np.asarray(devices), ("core",))
    in_specs = (PartitionSpec("core"),) * (n_params + n_outs)
    out_specs = (PartitionSpec("core"),) * n_outs
    donate = tuple(range(n_params, n_params + n_outs))
    sharded = jax.jit(
        shard_map(_body, mesh=mesh, in_specs=in_specs, out_specs=out_specs,
                  check_rep=False),
        donate_argnums=donate, keep_unused=True)
    _RUNNER_CACHE = (sharded, in_names, out_names, out_avals, zero_outs, n_params)
    return _RUNNER_CACHE


def _run_cached(in_maps):
    sharded, in_names, out_names, out_avals, zero_outs, n_params = _get_runner()
    concat_in = [
        np.concatenate([np.asarray(in_maps[c][name]) for c in range(NCORES)], axis=0)
        for name in in_names
    ]
    concat_zeros = [
        np.zeros((NCORES * z.shape[0], *z.shape[1:]), z.dtype) for z in zero_outs
    ]
    out_arrs = sharded(*concat_in, *concat_zeros)
    return [
        {
            name: np.asarray(out_arrs[i]).reshape(NCORES, *out_avals[i].shape)[c]
            for i, name in enumerate(out_names)
        }
        for c in range(NCORES)
    ]


def kernel(**inputs) -> np.ndarray:
    in_maps = _prep_maps(**inputs)
    results = _run_cached(in_maps)
    out = np.zeros((B_FULL, 2), np.float32)
    for c in range(NCORES):
        out[c * BB:(c + 1) * BB] = results[c]["out2"].T
    return out


# revision 9
# speedup vs baseline: 3.7701x; 1.1173x over previous
"""Trainium2 Bass kernel for nn_GammaModel (3-block Mamba-style model).

Sharding: data-parallel over batch. 8 cores x 4 samples each; all weights
replicated. Feature-major ([feature, token]) layout on device throughout:
 - all dense layers run on PE with the weight stationary,
 - the causal depthwise conv runs on PE as 8 PSUM-accumulated matmuls with
   host-diagonalized tap weights,
 - dA = exp(A[:,s] * delta) runs on ACT with per-partition scale,
 - B/C row broadcasts run on GPSIMD (partition_broadcast),
 - the selective scan itself is DVE tensor_tensor_scan (h = dA*h + dBu)
   along the free/time axis, one instruction per (state, sample).
"""

import sys

sys.path.insert(0, "/opt/trn_rl_repo")

import numpy as np
import ml_dtypes

from concourse import bacc, bass, mybir, tile
from concourse.bass_utils import run_bass_kernel_spmd

F32 = mybir.dt.float32
BF16 = mybir.dt.bfloat16
AF = mybir.ActivationFunctionType
ALU = mybir.AluOpType

# Model dims (hardcoded per problem spec)
NB = 3          # mamba blocks
B_FULL = 32     # full batch
NCORES = 8
BB = B_FULL // NCORES   # samples per core
L = 4096
T = BB * L      # tokens per core
DM = 32
DI = 128
DS = 12
DC = 8
DR = 2
CH = 512        # psum column chunk


def _build_nc():
    nc = bacc.Bacc(None, target_bir_lowering=False, debug=False)

    # ---- dram I/O ----
    xT_d = nc.dram_tensor("xT", (4, T), F32, kind="ExternalInput")
    fc0_wT_d = nc.dram_tensor("fc0_wT", (4, DM), F32, kind="ExternalInput")
    fc1_wT_d = nc.dram_tensor("fc1_wT", (DM, 2), F32, kind="ExternalInput")
    fc1_b_d = nc.dram_tensor("fc1_b", (2, 1), F32, kind="ExternalInput")
    lin_wT_d, lin_b_d, in_wT_d = [], [], []
    convd_d, conv_b_d, xproj_wT_d = [], [], []
    dt_wT_d, dt_b_d, A_d, Dp_d, out_wT_d = [], [], [], [], []
    for i in range(NB):
        lin_wT_d.append(nc.dram_tensor(f"lin_wT{i}", (DM, DM), F32, kind="ExternalInput"))
        lin_b_d.append(nc.dram_tensor(f"lin_b{i}", (DM, 1), F32, kind="ExternalInput"))
        in_wT_d.append(nc.dram_tensor(f"in_wT{i}", (DM, 2 * DI), F32, kind="ExternalInput"))
        convd_d.append(nc.dram_tensor(f"convd{i}", (DI, DC, DI), BF16, kind="ExternalInput"))
        conv_b_d.append(nc.dram_tensor(f"conv_b{i}", (DI, 1), F32, kind="ExternalInput"))
        xproj_wT_d.append(nc.dram_tensor(f"xproj_wT{i}", (DI, DR + 2 * DS), BF16, kind="ExternalInput"))
        dt_wT_d.append(nc.dram_tensor(f"dt_wT{i}", (DR, DI), BF16, kind="ExternalInput"))
        dt_b_d.append(nc.dram_tensor(f"dt_b{i}", (DI, 1), F32, kind="ExternalInput"))
        A_d.append(nc.dram_tensor(f"A{i}", (DI, DS), F32, kind="ExternalInput"))
        Dp_d.append(nc.dram_tensor(f"Dp{i}", (DI, 1), F32, kind="ExternalInput"))
        out_wT_d.append(nc.dram_tensor(f"out_wT{i}", (DI, DM), BF16, kind="ExternalInput"))
    out_d = nc.dram_tensor("out2", (2, BB), F32, kind="ExternalOutput")

    with tile.TileContext(nc) as tc:
        with (
            tc.tile_pool(name="w", bufs=1) as wp,
            tc.tile_pool(name="u", bufs=1) as up,
            tc.tile_pool(name="work", bufs=1) as pp,
            tc.tile_pool(name="small", bufs=2) as sp,
            tc.tile_pool(name="psum", bufs=1, space=bass.MemorySpace.PSUM) as psp,
        ):
            # ---- load weights (once) ----
            def wload(dram, shape, dtype, tag):
                t = wp.tile(shape, dtype, tag=tag)
                nc.sync.dma_start(t[:], dram[:])
                return t

            fc0_wT = wload(fc0_wT_d, (4, DM), F32, "fc0")
            fc1_wT = wload(fc1_wT_d, (DM, 2), F32, "fc1")
            fc1_b = wload(fc1_b_d, (2, 1), F32, "fc1b")
            lin_wT = [wload(lin_wT_d[i], (DM, DM), F32, f"linw{i}") for i in range(NB)]
            lin_b = [wload(lin_b_d[i], (DM, 1), F32, f"linb{i}") for i in range(NB)]
            in_wT = [wload(in_wT_d[i], (DM, 2 * DI), F32, f"inw{i}") for i in range(NB)]
            convd = [wload(convd_d[i], (DI, DC, DI), BF16, f"convd{i}") for i in range(NB)]
            conv_b = [wload(conv_b_d[i], (DI, 1), F32, f"convb{i}") for i in range(NB)]
            xproj_wT = [wload(xproj_wT_d[i], (DI, DR + 2 * DS), BF16, f"xpw{i}") for i in range(NB)]
            dt_wT = [wload(dt_wT_d[i], (DR, DI), BF16, f"dtw{i}") for i in range(NB)]
            dt_b = [wload(dt_b_d[i], (DI, 1), F32, f"dtb{i}") for i in range(NB)]
            A_t = [wload(A_d[i], (DI, DS), F32, f"A{i}") for i in range(NB)]
            Dp_t = [wload(Dp_d[i], (DI, 1), F32, f"Dp{i}") for i in range(NB)]
            out_wT = [wload(out_wT_d[i], (DI, DM), BF16, f"outw{i}") for i in range(NB)]

            u_a = nc.dram_tensor("u_dram_a", (DM, T), F32)
            u_b = nc.dram_tensor("u_dram_b", (DM, T), F32)
            ubufs = [u_a, u_b]

            # ---- embed: u0 = fc0_w_scaled @ xT  (+fc0_b folded == 0) ----
            for j in range(T // CH):
                xchunk = sp.tile((4, CH), F32, tag="xchunk")
                nc.sync.dma_start(xchunk[:], xT_d[:, j * CH:(j + 1) * CH])
                ps = psp.tile((DM, CH), F32, tag="pA")
                nc.tensor.matmul(ps[:], fc0_wT[:], xchunk[:])
                ustage = sp.tile((DM, CH), F32, tag="ustage")
                nc.scalar.copy(ustage[:], ps[:])
                nc.sync.dma_start(u_a[:, j * CH:(j + 1) * CH], ustage[:])

            # ---- blocks ----
            for i in range(NB):
                uin = ubufs[i % 2]
                uout = ubufs[(i + 1) % 2]
                for n in range(BB):
                    base = n * L
                    xhat = pp.tile((DI, L), BF16, tag="xhat")
                    zbuf = pp.tile((DI, L), BF16, tag="zbuf")
                    # lin -> tanh -> in_proj (x, z)
                    for j in range(L // CH):
                        gc = base + j * CH
                        lc = j * CH
                        uc = sp.tile((DM, CH), F32, tag="uc")
                        nc.sync.dma_start(uc[:], uin[:, gc:gc + CH])
                        lps = psp.tile((DM, CH), F32, tag="pA")
                        nc.tensor.matmul(lps[:], lin_wT[i][:], uc[:])
                        linc = sp.tile((DM, CH), F32, tag="linc")
                        nc.scalar.activation(linc[:], lps[:], AF.Tanh, bias=lin_b[i][:, 0:1])
                        xps = psp.tile((DI, CH), F32, tag="pX")
                        nc.tensor.matmul(xps[:], in_wT[i][:, 0:DI], linc[:])
                        zps = psp.tile((DI, CH), F32, tag="pZ")
                        nc.tensor.matmul(zps[:], in_wT[i][:, DI:2 * DI], linc[:])
                        nc.scalar.copy(xhat[:, lc:lc + CH], xps[:])
                        nc.vector.tensor_copy(zbuf[:, lc:lc + CH], zps[:])
                    # causal depthwise conv (PE, accumulated diag matmuls) + silu
                    xc = pp.tile((DI, L), BF16, tag="xc")
                    for j in range(L // CH):
                        lc = j * CH
                        cps = psp.tile((DI, CH), F32, tag="pC")
                        nc.tensor.matmul(cps[:], convd[i][:, DC - 1, :], xhat[:, lc:lc + CH],
                                         start=True, stop=False)
                        for k in range(DC - 1):
                            s = DC - 1 - k
                            last = (k == DC - 2)
                            if j == 0:
                                nc.tensor.matmul(cps[:, s:CH], convd[i][:, k, :],
                                                 xhat[:, 0:CH - s],
                                                 start=False, stop=last)
                            else:
                                nc.tensor.matmul(cps[:], convd[i][:, k, :],
                                                 xhat[:, lc - s:lc - s + CH],
                                                 start=False, stop=last)
                        nc.scalar.activation(xc[:, lc:lc + CH], cps[:], AF.Silu,
                                             bias=conv_b[i][:, 0:1])
                    # xproj -> dt/B/C rows
                    dtBC = pp.tile((DR + 2 * DS, L), BF16, tag="dtBC")
                    for j in range(L // CH):
                        lc = j * CH
                        pps = psp.tile((DR + 2 * DS, CH), F32, tag="pP")
                        nc.tensor.matmul(pps[:], xproj_wT[i][:], xc[:, lc:lc + CH])
                        nc.vector.tensor_copy(dtBC[:, lc:lc + CH], pps[:])
                    # delta = softplus(dt @ dt_w.T + dt_b) = ln(1 + exp(.))
                    # (no softplus table in this compiler build; args are tiny
                    #  so exp cannot overflow)
                    deltaT = pp.tile((DI, L), F32, tag="deltaT")
                    for j in range(L // CH):
                        lc = j * CH
                        dps = psp.tile((DI, CH), F32, tag="pD")
                        nc.tensor.matmul(dps[:], dt_wT[i][:], dtBC[0:DR, lc:lc + CH])
                        spe = sp.tile((DI, CH), F32, tag="spe")
                        nc.scalar.activation(spe[:], dps[:], AF.Exp,
                                             bias=dt_b[i][:, 0:1])
                        nc.vector.tensor_scalar_add(spe[:], spe[:], 1.0)
                        nc.scalar.activation(deltaT[:, lc:lc + CH], spe[:], AF.Ln)
                    # du = delta * x
                    du = pp.tile((DI, L), BF16, tag="du")
                    nc.vector.tensor_mul(du[:], deltaT[:], xc[:])
                    # selective scan over states
                    ybf = pp.tile((DI, L), BF16, tag="ybf")
                    for s in range(DS):
                        dA = pp.tile((DI, L), BF16, tag="dA")
                        nc.scalar.activation(dA[:], deltaT[:], AF.Exp,
                                             scale=A_t[i][:, s:s + 1])
                        browB = sp.tile((1, L), BF16, tag="browB")
                        nc.sync.dma_start(browB[:], dtBC[DR + s:DR + s + 1, :])
                        bcB = pp.tile((DI, L), BF16, tag="bcB")
                        nc.gpsimd.partition_broadcast(bcB[:], browB[0:1, :])
                        dBu = pp.tile((DI, L), BF16, tag="dBu")
                        nc.vector.tensor_mul(dBu[:], du[:], bcB[:])
                        h = pp.tile((DI, L), BF16, tag="h")
                        nc.vector.tensor_tensor_scan(h[:], dA[:], dBu[:], 0.0,
                                                     ALU.mult, ALU.add)
                        browC = sp.tile((1, L), BF16, tag="browC")
                        nc.sync.dma_start(browC[:], dtBC[DR + DS + s:DR + DS + s + 1, :])
                        bcC = pp.tile((DI, L), BF16, tag="bcC")
                        nc.gpsimd.partition_broadcast(bcC[:], browC[0:1, :])
                        if s == 0:
                            nc.vector.tensor_mul(ybf[:], h[:], bcC[:])
                        else:
                            hC = pp.tile((DI, L), BF16, tag="hC")
                            nc.vector.tensor_mul(hC[:], h[:], bcC[:])
                            nc.vector.tensor_add(ybf[:], ybf[:], hC[:])
                    # y = (x*Dp + y) * silu(z);  out = relu(out_w @ y)
                    y1 = pp.tile((DI, L), BF16, tag="y1")
                    nc.vector.scalar_tensor_tensor(y1[:], xc[:], Dp_t[i][:, 0:1], ybf[:],
                                                   ALU.mult, ALU.add)
                    sz = pp.tile((DI, L), BF16, tag="sz")
                    nc.scalar.activation(sz[:], zbuf[:], AF.Silu)
                    y2 = pp.tile((DI, L), BF16, tag="y2")
                    nc.vector.tensor_mul(y2[:], y1[:], sz[:])
                    for j in range(L // CH):
                        lc = j * CH
                        ops = psp.tile((DM, CH), F32, tag="pA")
                        nc.tensor.matmul(ops[:], out_wT[i][:], y2[:, lc:lc + CH])
                        ustage = sp.tile((DM, CH), F32, tag="ustage")
                        nc.scalar.activation(ustage[:], ops[:], AF.Relu)
                        nc.sync.dma_start(uout[:, base + lc:base + lc + CH], ustage[:])
            # ---- head: fc1 on last token of each sample ----
            ufin = ubufs[NB % 2]
            lastc = sp.tile((DM, BB), F32, tag="lastc")
            for n in range(BB):
                nc.sync.dma_start(lastc[:, n:n + 1], ufin[:, (n + 1) * L - 1:(n + 1) * L])
            fps = psp.tile((2, BB), F32, tag="pP")
            nc.tensor.matmul(fps[:], fc1_wT[:], lastc[:])
            outsb = sp.tile((2, BB), F32, tag="outsb")
            nc.scalar.activation(outsb[:], fps[:], AF.Relu, bias=fc1_b[:, 0:1])
            nc.sync.dma_start(out_d[:], outsb[:])

    nc.compile()
    return nc


_NC_CACHE = None


def _get_nc():
    global _NC_CACHE
    if _NC_CACHE is None:
        _NC_CACHE = _build_nc()
    return _NC_CACHE


def _prep_maps(x, fc0_w, fc0_b, lin_w, lin_b, in_w, conv_w, conv_b, xproj_w,
               dt_w, dt_b, A_log, D, out_w, fc1_w, fc1_b):
    f32 = np.float32
    bf16 = ml_dtypes.bfloat16
    start_max = np.max(np.asarray(x[:, :, 2], f32))
    scale = np.array([1.0 / 255.0, 1.0 / 255.0, 1.0 / start_max, 1.0], f32)
    fc0_wT = (np.asarray(fc0_w, f32) * scale[None, :]).T.copy()  # [4, 32]

    common = {
        "fc0_wT": fc0_wT.astype(f32),
        "fc1_wT": np.asarray(fc1_w, f32).T.copy(),
        "fc1_b": np.asarray(fc1_b, f32).reshape(2, 1),
    }
    for i in range(NB):
        common[f"lin_wT{i}"] = np.asarray(lin_w[i], f32).T.copy()
        common[f"lin_b{i}"] = np.asarray(lin_b[i], f32).reshape(DM, 1)
        common[f"in_wT{i}"] = np.asarray(in_w[i], f32).T.copy()
        cd = np.zeros((DI, DC, DI), f32)
        cw = np.asarray(conv_w[i], f32)
        for k in range(DC):
            cd[np.arange(DI), k, np.arange(DI)] = cw[:, k]
        common[f"convd{i}"] = cd.astype(bf16)
        common[f"conv_b{i}"] = np.asarray(conv_b[i], f32).reshape(DI, 1)
        common[f"xproj_wT{i}"] = np.asarray(xproj_w[i], f32).T.copy().astype(bf16)
        common[f"dt_wT{i}"] = np.asarray(dt_w[i], f32).T.copy().astype(bf16)
        common[f"dt_b{i}"] = np.asarray(dt_b[i], f32).reshape(DI, 1)
        common[f"A{i}"] = (-np.exp(np.asarray(A_log[i], f32))).astype(f32)
        common[f"Dp{i}"] = np.asarray(D[i], f32).reshape(DI, 1)
        common[f"out_wT{i}"] = np.asarray(out_w[i], f32).T.copy().astype(bf16)

    xf = np.asarray(x, f32)
    in_maps = []
    for c in range(NCORES):
        xc = xf[c * BB:(c + 1) * BB]          # [BB, L, 4]
        xTc = xc.reshape(BB * L, 4).T.copy()  # [4, T]
        m = dict(common)
        m["xT"] = np.ascontiguousarray(xTc)
        in_maps.append(m)
    return in_maps


_RUNNER_CACHE = None


def _get_runner():
    """Build (once) a cached jitted SPMD runner equivalent to
    bass2jax.run_bass_via_pjrt, so repeat kernel() calls skip retracing."""
    global _RUNNER_CACHE
    if _RUNNER_CACHE is not None:
        return _RUNNER_CACHE
    import jax
    from jax.sharding import Mesh, PartitionSpec
    from jax.experimental.shard_map import shard_map
    from concourse import bass2jax, mybir as _mybir

    nc = _get_nc()
    bass2jax.install_neuronx_cc_hook()
    partition_name = nc.partition_id_tensor.name if nc.partition_id_tensor else None
    in_names, out_names, out_avals, zero_outs = [], [], [], []
    for alloc in nc.m.functions[0].allocations:
        if not isinstance(alloc, _mybir.MemoryLocationSet):
            continue
        name = alloc.memorylocations[0].name
        if alloc.kind == "ExternalInput":
            if name != partition_name:
                in_names.append(name)
        elif alloc.kind == "ExternalOutput":
            shape = tuple(alloc.tensor_shape)
            dtype = _mybir.dt.np(alloc.dtype)
            out_avals.append(jax.core.ShapedArray(shape, dtype))
            out_names.append(name)
            zero_outs.append(np.zeros(shape, dtype))
    n_params = len(in_names)
    n_outs = len(out_avals)
    all_in = list(in_names) + list(out_names)
    if partition_name is not None:
        all_in.append(partition_name)

    def _body(*args):
        operands = list(args)
        if partition_name is not None:
            operands.append(bass2jax.partition_id_tensor())
        outs = bass2jax._bass_exec_p.bind(
            *operands,
            out_avals=tuple(out_avals),
            in_names=tuple(all_in),
            out_names=tuple(out_names),
            lowering_input_output_aliases=(),
            sim_require_finite=True,
            sim_require_nnan=True,
            nc=nc,
        )
        return tuple(outs)

    devices = jax.devices()[:NCORES]
    mesh = Mesh(np.asarray(devices), ("core",))
    in_specs = (PartitionSpec("core"),) * (n_params + n_outs)
    out_specs = (PartitionSpec("core"),) * n_outs
    donate = tuple(range(n_params, n_params + n_outs))
    sharded = jax.jit(
        shard_map(_body, mesh=mesh, in_specs=in_specs, out_specs=out_specs,
                  check_rep=False),
        donate_argnums=donate, keep_unused=True)
    _RUNNER_CACHE = (sharded, in_names, out_names, out_avals, zero_outs, n_params)
    return _RUNNER_CACHE


def _run_cached(in_maps):
    sharded, in_names, out_names, out_avals, zero_outs, n_params = _get_runner()
    concat_in = [
        np.concatenate([np.asarray(in_maps[c][name]) for c in range(NCORES)], axis=0)
        for name in in_names
    ]
    concat_zeros = [
        np.zeros((NCORES * z.shape[0], *z.shape[1:]), z.dtype) for z in zero_outs
    ]
    out_arrs = sharded(*concat_in, *concat_zeros)
    return [
        {
            name: np.asarray(out_arrs[i]).reshape(NCORES, *out_avals[i].shape)[c]
            for i, name in enumerate(out_names)
        }
        for c in range(NCORES)
    ]


def kernel(**inputs) -> np.ndarray:
    in_maps = _prep_maps(**inputs)
    results = _run_cached(in_maps)
    out = np.zeros((B_FULL, 2), np.float32)
    for c in range(NCORES):
        out[c * BB:(c + 1) * BB] = results[c]["out2"].T
    return out
